# revision 15
# baseline (speedup 1.0000x reference)
"""Trainium2 Bass kernel for nn_Attention_39934605918652.

res[b] = W0 @ x0[b] + sum_{n=1..N-1} W2 @ tanh(W1a @ x0[b] + W1b @ x[b,n])

Key algebraic optimization: W2 does not depend on n, so
    sum_n W2 @ tanh(...) = W2 @ (sum_n tanh(...))
which removes the second big matmul (only a [B,H]x[H,F] remains).

Sharding: data-parallel over batch B=128 across 8 cores (16 batches/core),
weights replicated. No collectives.

The dominant [F=512]-contraction matmul runs in fp8 e4m3 DoubleRow mode
(213ns per 512-col matmul warm = 512 cycles @2.4GHz streaming 2 packed
rhs cols/cycle; 2x bf16 FLOPs via 256-deep contraction). W1b is
host-scaled by 32 so its N(0, 1/1024) entries use the e4m3 range; the
tanh compensates via the ACT scale=1/32 immediate.

v20 changes (from trace analysis of the 75.2us v19 baseline):
  - DMA issue is SP-queue rate-limited (~650ns per DMA_DIRECT2D): merge
    to 11 host-packed loads in strict first-need order (x0, w1a, w1b,
    xi q0, xi q1, bmask, w0, xi q2, xi q3, w2). w1a first => phase 1
    runs ~10-13us instead of 16-19us.
  - Phase 1 rework: h0T via 8 [16,512] matmuls (+8 concurrent
    tile_position=(0,32) duplicates for the row-32 bias-burst replica),
    then h0 [128h,b]-layout via 8 one-hot K=16 transpose-matmuls
    against a host identity, replacing v19's 32 tiny matmuls (saves
    ~4us PE and pulls the first ACT call ~5us earlier).
  - S4 bias one-hot matmuls run as 2-tile row-group bursts
    (tile_position=(0,0)/(32,0)): concurrent in the PE array, ~halving
    the 0.63us/tile bias cost. Requires h0T+bmask replicas at
    partitions 32-47.
  - Tiles scheduled as (h, q-pair)s; per-pair class (s1 = 4 fused-bias
    ACT calls, s4 = PE bias + 1 big ACT call) balances PE vs ACT:
    ACT small call ~590ns, big 1024-col ~1040ns, DVE reduce ~1210ns.

All DRAM tensors are host-packed so every SBUF tile loads with ONE
contiguous dma_start:
  xiQ4  [4*128, 4096] fp8   row q*128+p, col fp*2048+i*1024+c
  w1bQ2 [128, 4096]   fp8   (= 32*W1b.T, DoubleRow pair layout)
  x0T   [128, 4*16]   fp16  host-packed f-chunks side by side
  x0Q8  [128, 4*16]   fp8   same, for the fp8 W0-term matmuls
  w1a4  [128, 4096]   fp16  (= W1a.T, f-chunks side by side)
  w2Q4  [128, 4096]   fp16  h-tile pairs side by side (= W2.T regrouped)
  w0Q   [128, 2048]   fp8   f-chunks side by side (= W0.T regrouped)
  bmask [48, 4*1024]  fp16  one-hot bias mask; rows 32-47 replicate 0-15
  id16  [16, 16]      fp16  identity (h0 layout transpose)
Output res [BL=16, F=512] per core (batch-major); host concatenates.
"""

import os
import numpy as np
from contextlib import ExitStack

import concourse.bass as bass
import concourse.tile as tile
from concourse import bacc, mybir
from concourse.bass_utils import run_bass_kernel_spmd

N_CORES = 8
B, N, F, H = 128, 256, 512, 1024
BL = B // N_CORES          # 16 batches per core
NI = N - 1                 # 255 real columns per batch
NP = 256                   # padded columns per batch
NF = F // 128              # 4 f-chunks
FP = 2                     # 2 f-pair chunks (256 rows each, DoubleRow)
NH = H // 128              # 8 h-tiles
QUADS = BL // 4            # 4 batch-quads; per quad psum tile [128, 4*256]
QW = 4 * NP                # 1024 columns per quad
WSCALE = 32.0              # host-side W1b/bias scale (ACT scale=1/32)

F32 = mybir.dt.float32
BF16 = mybir.dt.bfloat16
F16 = mybir.dt.float16
F8 = mybir.dt.float8e4
DR = mybir.MatmulPerfMode.DoubleRow

# Knobs (sweepable on hw):
#  KB_NS1: number of s1 PAIRS (2 tiles each). Rest are s4 pairs.
#  KB_BURST: 1 = s4 bias matmuls as 2-tile row-group bursts; 0 = serial.
#  KB_DUP: 1 = h0T replica via concurrent tile_position=(0,32) phase1b
#          duplicate; 0 = serial second pass.
#  KB_NGH: consumes prefaced by a GpSimd halving add (measured 1154ns on
#          hw per tile = 0.37 eff), then a half-width DVE reduce.
#  KB_WARM: dummy [128,128] f32 matmuls (426ns each) to hold the PE
#          pstate ramp until the first real operands land.
#  KB_PPB: main PSUM pool bufs ([128,1024] f32 slots, 2 banks each).
NS1P = int(os.environ.get("KB_NS1", "8"))
BURST = int(os.environ.get("KB_BURST", "1"))
DUP = int(os.environ.get("KB_DUP", "1"))
NGH = int(os.environ.get("KB_NGH", "0"))
WARM_N = int(os.environ.get("KB_WARM", "12"))
PPB = int(os.environ.get("KB_PPB", "4"))
TAIL_S4 = int(os.environ.get("KB_TAIL", "3"))
ITB = int(os.environ.get("KB_ITB", "8"))


def _build_kernel():
    nc = bacc.Bacc(
        "TRN2", target_bir_lowering=False, debug=False, num_devices=N_CORES
    )

    xiQ4 = nc.dram_tensor("xiQ4", [QUADS * 128, 4096], F8, kind="ExternalInput").ap()
    w1bQ2 = nc.dram_tensor("w1bQ2", [128, 4096], F8, kind="ExternalInput").ap()
    x0T = nc.dram_tensor("x0T", [128, NF * BL], F16, kind="ExternalInput").ap()
    x0Q8 = nc.dram_tensor("x0Q8", [128, NF * BL], F8, kind="ExternalInput").ap()
    w1a4 = nc.dram_tensor("w1a4", [128, 4096], F16, kind="ExternalInput").ap()
    w2Q4 = nc.dram_tensor("w2Q4", [128, 4096], F16, kind="ExternalInput").ap()
    w0Q = nc.dram_tensor("w0Q", [128, 2048], F8, kind="ExternalInput").ap()
    bmaskT = nc.dram_tensor("bmaskT", [48, QUADS * 1024], F16, kind="ExternalInput").ap()
    id16 = nc.dram_tensor("id16", [16, 16], F16, kind="ExternalInput").ap()
    res = nc.dram_tensor("res", [BL, F], F32, kind="ExternalOutput").ap()

    with tile.TileContext(nc) as tc:
        with ExitStack() as ctx:
            _kernel_body(
                ctx, tc, xiQ4, w1bQ2, x0T, x0Q8, w1a4, w2Q4, w0Q, bmaskT,
                id16, res
            )

    nc.compile()
    return nc


def _kernel_body(ctx, tc, xiQ4, w1bQ2, x0T, x0Q8, w1a4, w2Q4, w0Q, bmaskT,
                 id16, res):
    nc = tc.nc
    Tanh = mybir.ActivationFunctionType.Tanh

    wpool = ctx.enter_context(tc.tile_pool(name="weights", bufs=1))

    def load_rows(name, dram, shape, dt):
        t = wpool.tile(shape, dt, tag=name, name=name)
        flat = t[:] if len(shape) == 2 else t[:].rearrange(
            "p a b c -> p (a b c)" if len(shape) == 4 else "p a b -> p (a b)"
        )
        nc.sync.dma_start(flat, dram[0 : shape[0], :])
        return t

    def load_rows_at(name, dram, r0, shape, dt):
        t = wpool.tile(shape, dt, tag=name, name=name)
        flat = t[:].rearrange("p a b c -> p (a b c)")
        nc.sync.dma_start(flat, dram[r0 : r0 + 128, :])
        return t

    # ---- DMA issue order = first-need order. Each DMA_DIRECT2D costs
    # ~650ns serialized on the SP queue, so the count and order ARE the
    # lead-in: w1a gates phase 1 (which gates every ACT call), xi q0
    # gates the first tile's matmuls.
    x0_all = load_rows("x0", x0T, [128, NF * BL], F16)
    x08_all = load_rows("x08", x0Q8, [128, NF * BL], F8)
    id_sb = wpool.tile([16, 16], F16, tag="id16", name="id16")
    nc.sync.dma_start(id_sb[:], id16[:, :])
    w1a_all = load_rows("w1a", w1a4, [128, 4, 1024], F16)
    w1b2 = load_rows("w1b", w1bQ2, [128, 2, 2, 1024], F8)
    xi_sb = [None] * QUADS
    for q in (0, 1):
        xi_sb[q] = load_rows_at(f"xi_{q}", xiQ4, q * 128, [128, 2, 2, 1024], F8)
    bmask_sb = wpool.tile([48, QUADS * 1024], F16, tag="bmask", name="bmask")
    nc.sync.dma_start(bmask_sb[:], bmaskT[:, :])
    w0_sb = load_rows("w0", w0Q, [128, 2048], F8)
    for q in (2, 3):
        xi_sb[q] = load_rows_at(f"xi_{q}", xiQ4, q * 128, [128, 2, 2, 1024], F8)
    w2_all = load_rows("w2", w2Q4, [128, 4, 1024], F16)

    x0_sb = [x0_all[:, f * BL : (f + 1) * BL] for f in range(NF)]
    x08_sb = [x08_all[:, f * BL : (f + 1) * BL] for f in range(NF)]

    def w2_slice(h):
        return w2_all[:, h // 2, (h % 2) * 512 : (h % 2 + 1) * 512]

    # h0T2: rows 0-15 = h0T [b, h], rows 32-47 = replica for row-group
    # bias bursts. h0_all: [128, h*16+b] fp16 for the ACT bias port.
    h0T2_sb = wpool.tile([48, H], F16, tag="h0T", name="h0T")
    h0_all = wpool.tile([128, NH * BL], F16, tag="h0a", name="h0a")
    S_sb = [
        wpool.tile([128, BL], F16, tag=f"S_{h}", name=f"S_{h}")
        for h in range(NH)
    ]

    # One PSUM pool; every tile shares the tag so slots recycle.
    # Slot = [128, 4*NP] f32 = 2 banks; PPB slots = the full 8 banks.
    ppool = ctx.enter_context(tc.tile_pool(name="ps", bufs=PPB, space="PSUM"))
    itpool = ctx.enter_context(tc.tile_pool(name="it", bufs=ITB))

    # ---- Phase 0: PE warm-up to ride the pstate ramp until w1a lands.
    if WARM_N:
        wz = wpool.tile([128, 128], F32, tag="warmz", name="warmz")
        nc.vector.memset(wz[:], 0.0)
        pw = ppool.tile([128, 128], F32, tag="ps", name="pwarm")
        for _ in range(WARM_N):
            nc.tensor.matmul(pw[:], wz[:], wz[:], start=True, stop=True)

    # ---- Phase 0b: preload the tanh ACT table during the DMA lead-in
    tiny = wpool.tile([128, 1], F32, tag="tiny", name="tiny")
    nc.vector.memset(tiny[:], 0.0)
    nc.scalar.activation(tiny[:], tiny[:], Tanh)

    # ---- Phase 1: h0T[b,h] = sum_f x0[b,f] W1a[h,f] via 8 [16,512]
    # matmuls (f-outer so each starts as its w1a chunk lands), plus a
    # concurrent col-group-32 duplicate for the burst replica. Then
    # h0[h*128+p, b] via 8 one-hot K=16 matmuls against id16.
    def phase1():
        ph = ppool.tile([48, H], F32, tag="ps", name="ph_h0T")
        for f in range(NF):
            for hb in range(2):
                nc.tensor.matmul(
                    ph[0:BL, hb * 512 : (hb + 1) * 512],
                    x0_sb[f],
                    w1a_all[:, f, hb * 512 : (hb + 1) * 512],
                    start=(f == 0),
                    stop=(f == NF - 1),
                )
                if DUP:
                    nc.tensor.matmul(
                        ph[32:48, hb * 512 : (hb + 1) * 512],
                        x0_sb[f],
                        w1a_all[:, f, hb * 512 : (hb + 1) * 512],
                        start=(f == 0),
                        stop=(f == NF - 1),
                        tile_position=(0, 32),
                        skip_group_check=True,
                    )
        with nc.allow_low_precision(reason="h0T feeds fp16 bias matmul"):
            nc.vector.tensor_copy(h0T2_sb[0:BL, :], ph[0:BL, :])
            if DUP:
                nc.vector.tensor_copy(h0T2_sb[32:48, :], ph[32:48, :])
        if not DUP:
            for f in range(NF):
                for hb in range(2):
                    nc.tensor.matmul(
                        ph[32:48, hb * 512 : (hb + 1) * 512],
                        x0_sb[f],
                        w1a_all[:, f, hb * 512 : (hb + 1) * 512],
                        start=(f == 0),
                        stop=(f == NF - 1),
                    )
            with nc.allow_low_precision(reason="h0T replica"):
                nc.vector.tensor_copy(h0T2_sb[32:48, :], ph[32:48, :])
        # h0 layout flip: psum[128, h*16+b] = h0T[0:16, h*128+p].T @ id16
        ph0 = ppool.tile([128, NH * BL], F32, tag="ps", name="ph_h0")
        for h in range(NH):
            nc.tensor.matmul(
                ph0[:, h * BL : (h + 1) * BL],
                h0T2_sb[0:BL, h * 128 : (h + 1) * 128],
                id_sb[:],
                start=True,
                stop=True,
            )
        with nc.allow_low_precision(reason="h0 bias in fp16 like h0T"):
            nc.vector.tensor_copy(h0_all[:], ph0[:])

    # ---- Phase 3: epilogue res = W0 x0 + W2 S, accumulated in SBUF.
    rt_acc = wpool.tile([BL, F], F32, tag="rt", name="rt_acc")

    def epilogue_w0():
        pw = ppool.tile([BL, F], F32, tag="ps", name="po_w0")
        for f in range(NF):
            nc.tensor.matmul(
                pw[:],
                x08_sb[f],
                w0_sb[:, f * 512 : (f + 1) * 512],
                start=(f == 0),
                stop=(f == NF - 1),
            )
        nc.vector.tensor_copy(rt_acc[:], pw[:])

    def epilogue_s_group(hs, name):
        pg = ppool.tile([BL, F], F32, tag="ps", name=name)
        for i, h in enumerate(hs):
            nc.tensor.matmul(
                pg[:], S_sb[h][:], w2_slice(h),
                start=(i == 0), stop=(i == len(hs) - 1),
            )
        nc.vector.tensor_add(rt_acc[:], rt_acc[:], pg[:])

    # ---- Phase 2: hi matmul (fp8 DoubleRow) + bias + tanh + reduce ----
    def consume(h, q, pb, cls, red):
        it = itpool.tile([128, 4 * NP], BF16, tag="it", name=f"it_{h}_{q}")
        s1ish = cls in ("s1", "s1a")
        nb = NP if cls == "s4" else NI
        with nc.allow_low_precision(
            reason="S accumulated in 16-bit to feed the 16-bit output matmul"
        ):
            if s1ish:
                for bl in range(4):
                    b = q * 4 + bl
                    acc = S_sb[h][:, b : b + 1] if cls == "s1a" else None
                    nc.scalar.activation(
                        it[:, bl * NP : bl * NP + NI],
                        pb[:, bl * NP : bl * NP + NI],
                        Tanh,
                        bias=h0_all[:, h * BL + b : h * BL + b + 1],
                        scale=1.0 / WSCALE,
                        accum_out=acc,
                    )
                if cls == "s1a":
                    return
            else:
                # S4: bias already in PSUM (one-hot matmul, pad col exact 0
                # since bmask zeroes it and tanh(0)=0): one big tanh call.
                nc.scalar.activation(it[:], pb[:], Tanh, scale=1.0 / WSCALE)
            scol = S_sb[h][:, q * 4 : (q + 1) * 4]
            view = it[:].rearrange("p (b n) -> p b n", b=4)
            if red == "gph":
                hb = nb // 2
                nc.gpsimd.tensor_add(
                    view[:, :, :hb],
                    view[:, :, :hb],
                    view[:, :, nb - hb : nb],
                )
                nc.vector.reduce_sum(
                    scol, view[:, :, : nb - hb], axis=mybir.AxisListType.X
                )
            else:
                nc.vector.reduce_sum(
                    scol, view[:, :, :nb], axis=mybir.AxisListType.X
                )

    def mm_main(pb, h, q, s1ish):
        # 2 DoubleRow matmuls per 512-col block: fpair 0 starts, fpair 1
        # accumulates; S4 groups stay open for the bias matmul.
        for bk in range(2):
            out = pb[:, bk * 512 : (bk + 1) * 512]
            for fp in range(FP):
                nc.tensor.matmul(
                    out,
                    w1b2[:, fp, :, h * 128 : (h + 1) * 128],
                    xi_sb[q][:, fp, :, bk * 512 : (bk + 1) * 512],
                    start=(fp == 0),
                    stop=(fp == FP - 1) and s1ish,
                    perf_mode=DR,
                )

    def mm_bias(pb, h, q, row):
        # One-hot bias matmul; row-group `row` (0 or 32) lets two of
        # these run concurrently in the PE array.
        for bk in range(2):
            nc.tensor.matmul(
                pb[:, bk * 512 : (bk + 1) * 512],
                h0T2_sb[row : row + BL, h * 128 : (h + 1) * 128],
                bmask_sb[row : row + BL,
                         q * 1024 + bk * 512 : q * 1024 + (bk + 1) * 512],
                start=False,
                stop=True,
                tile_position=(row, 0) if BURST else None,
            )

    def mm_bias_burst(pb0, pb1, h, q0, q1):
        # Interleave the two tiles' bias mms bk-wise so the row-0 and
        # row-32 instructions sit back-to-back and overlap in the array.
        for bk in range(2):
            for row, pb, q in ((0, pb0, q0), (32, pb1, q1)):
                nc.tensor.matmul(
                    pb[:, bk * 512 : (bk + 1) * 512],
                    h0T2_sb[row : row + BL, h * 128 : (h + 1) * 128],
                    bmask_sb[row : row + BL,
                             q * 1024 + bk * 512 : q * 1024 + (bk + 1) * 512],
                    start=False,
                    stop=True,
                    tile_position=(row, 0),
                )

    # ---- Schedule: 16 (h, q-pair)s. Wave 0 ascends h on quads (0,1);
    # wave 1 descends h on quads (2,3) so S[7..4] complete early and
    # their epilogue group issues mid-stream.
    pairs = []
    for wave in range(2):
        hs = range(NH) if wave == 0 else range(NH - 1, -1, -1)
        for h in hs:
            pairs.append((h, 2 * wave, 2 * wave + 1))

    # Deficit-spread pair classes; last TAIL_S4 pairs forced s4 (single
    # big tanh call drains the ACT pipeline fastest).
    counts = {"s1": NS1P, "s4": 16 - NS1P}
    labels = []
    used = {k: 0 for k in counts}
    for pos in range(16):
        opts = [k for k in counts if used[k] < counts[k]]
        if pos == 0 and counts["s1"]:
            # pair 0 stays s1: its ACT needs only h0_all, not the longer
            # h0T-copy -> bias-matmul chain (lead-in critical path).
            opts = ["s1"]
        elif pos >= 16 - TAIL_S4 and used["s4"] < counts["s4"]:
            opts = ["s4"]
        pick = max(opts, key=lambda k: counts[k] * (pos + 1) / 16 - used[k])
        used[pick] += 1
        labels.append(pick)
    nred = sum(2 for l in labels)
    rcounts = {"gph": min(NGH, nred)}
    rcounts["plain"] = nred - rcounts["gph"]
    rlabels = []
    rused = {k: 0 for k in rcounts}
    for pos in range(nred):
        opts = [k for k in rcounts if rused[k] < rcounts[k]]
        if pos >= nred - 4 and rused["plain"] < rcounts["plain"]:
            opts = ["plain"]
        pick = max(opts, key=lambda k: rcounts[k] * (pos + 1) / nred - rused[k])
        rused[pick] += 1
        rlabels.append(pick)

    phase1()

    for pos, (h, qa, qb) in enumerate(pairs):
        cls = labels[pos]
        reda, redb = rlabels[2 * pos], rlabels[2 * pos + 1]
        pba = ppool.tile([128, 4 * NP], F32, tag="ps", name=f"pb_{h}_{qa}")
        pbb = ppool.tile([128, 4 * NP], F32, tag="ps", name=f"pb_{h}_{qb}")
        # Interleaved mains (fp-outer, qa/qb alternating) so both tiles
        # finish together and the 4 bias matmuls sit adjacent in the PE
        # stream, where the row-0/row-32 pairs overlap in the array.
        for fp in range(FP):
            for bk in range(2):
                for pb, q in ((pba, qa), (pbb, qb)):
                    nc.tensor.matmul(
                        pb[:, bk * 512 : (bk + 1) * 512],
                        w1b2[:, fp, :, h * 128 : (h + 1) * 128],
                        xi_sb[q][:, fp, :, bk * 512 : (bk + 1) * 512],
                        start=(fp == 0),
                        stop=(fp == FP - 1) and cls != "s4",
                        perf_mode=DR,
                    )
        if cls == "s4":
            mm_bias_burst(pba, pbb, h, qa, qb)
        consume(h, qa, pba, cls, reda)
        consume(h, qb, pbb, cls, redb)
        if pos == 5:
            epilogue_w0()
        if pos == 12:
            epilogue_s_group([7, 6, 5, 4], "po_sA")
        if pos == 14:
            epilogue_s_group([3, 2], "po_sB1")

    epilogue_s_group([1, 0], "po_sB2")
    nc.sync.dma_start(res[:], rt_acc[:])


_NC_CACHE = {}


def _get_nc():
    key = ("v23d", NS1P, BURST, DUP, NGH, WARM_N, PPB, TAIL_S4, ITB)
    if key not in _NC_CACHE:
        _NC_CACHE[key] = _build_kernel()
    return _NC_CACHE[key]


def _make_in_maps(x, W1, W2, W0):
    import ml_dtypes

    f8 = ml_dtypes.float8_e4m3
    x = np.ascontiguousarray(np.asarray(x, dtype=np.float32))
    W1 = np.asarray(W1, dtype=np.float32)
    W2 = np.asarray(W2, dtype=np.float32)
    W0 = np.asarray(W0, dtype=np.float32)

    w1aT = np.ascontiguousarray(W1[:, :F].T).astype(np.float16)       # [F, H]
    w1a4 = np.ascontiguousarray(
        w1aT.reshape(NF, 128, H).transpose(1, 0, 2).reshape(128, NF * H)
    )
    w1bT = (W1[:, F:].T * WSCALE).astype(f8)                          # [F, H]
    # DoubleRow pair layout: [128, fp*2048 + i*1024 + h]
    w1bQ2 = np.ascontiguousarray(
        w1bT.reshape(FP, 2, 128, H).transpose(2, 0, 1, 3).reshape(128, 4 * H)
    )
    w2T = np.ascontiguousarray(W2.T).astype(np.float16)               # [H, F]
    w2Q4 = np.ascontiguousarray(
        w2T.reshape(NF, 2, 128, F).transpose(2, 0, 1, 3).reshape(128, NF * 2 * F)
    )
    w0T = np.ascontiguousarray(W0.T).astype(f8)                       # [F, F]
    w0Q = np.ascontiguousarray(
        w0T.reshape(NF, 128, F).transpose(1, 0, 2).reshape(128, NF * F)
    )

    # bmask[r, q*1024 + b*256 + n] = WSCALE iff r%32 == q*4+b and n != 255;
    # rows 32-47 replicate rows 0-15 for the row-group bias bursts.
    bmask = np.zeros((48, QUADS, 4, NP), dtype=np.float16)
    for qq in range(QUADS):
        for bb in range(4):
            bmask[qq * 4 + bb, qq, bb, :NI] = WSCALE
            bmask[32 + qq * 4 + bb, qq, bb, :NI] = WSCALE
    bmask = bmask.reshape(48, QUADS * 1024)

    id16 = np.eye(16, dtype=np.float16)

    in_maps = []
    for i in range(N_CORES):
        xc = x[i * BL : (i + 1) * BL]               # [BL, N, F]
        # packed [128, NF*BL]: row p, block f holds x0T[f*128+p, :]
        x0p = np.ascontiguousarray(
            xc[:, 0, :].T.reshape(NF, 128, BL).transpose(1, 0, 2).reshape(128, NF * BL)
        )
        pad = np.zeros((BL, NP, F), dtype=np.float32)
        pad[:, :NI, :] = xc[:, 1:, :]
        xiT = pad.reshape(BL * NP, F).T.astype(f8)  # [F, BL*NP]
        # row q*128+p, col fp*2048 + i*1024 + c
        xiQ4 = np.ascontiguousarray(
            xiT.reshape(FP, 2, 128, QUADS, QW)
            .transpose(3, 2, 0, 1, 4)
            .reshape(QUADS * 128, 4 * QW)
        )
        in_maps.append(
            {
                "xiQ4": xiQ4,
                "x0T": x0p.astype(np.float16),
                "x0Q8": x0p.astype(f8),
                "w1bQ2": w1bQ2,
                "w1a4": w1a4,
                "w2Q4": w2Q4,
                "w0Q": w0Q,
                "bmaskT": bmask,
                "id16": id16,
            }
        )
    return in_maps


def _gather(results):
    out = np.empty((B, F), dtype=np.float32)
    for i in range(N_CORES):
        out[i * BL : (i + 1) * BL] = results[i]["res"]
    return out


def kernel(x, W1, W2, W0):
    nc = _get_nc()
    in_maps = _make_in_maps(x, W1, W2, W0)
    res = run_bass_kernel_spmd(nc, in_maps, list(range(N_CORES)))
    return _gather(res.results)


def kernel_profiled(x, W1, W2, W0, **trace_kwargs):
    """Like kernel() but with NTFF profiling; returns (out, exec_time_ns)."""
    nc = _get_nc()
    in_maps = _make_in_maps(x, W1, W2, W0)
    res = run_bass_kernel_spmd(
        nc, in_maps, list(range(N_CORES)), trace=True, **trace_kwargs
    )
    return _gather(res.results), res.exec_time_ns


# revision 18
# speedup vs baseline: 1.0505x; 1.0505x over previous
"""Trainium2 Bass kernel for nn_Attention_39934605918652.

res[b] = W0 @ x0[b] + sum_{n=1..N-1} W2 @ tanh(W1a @ x0[b] + W1b @ x[b,n])

Key algebraic optimization: W2 does not depend on n, so
    sum_n W2 @ tanh(...) = W2 @ (sum_n tanh(...))
which removes the second big matmul (only a [B,H]x[H,F] remains).

Sharding: data-parallel over batch B=128 across 8 cores (16 batches/core),
weights replicated. No collectives.

The dominant [F=512]-contraction matmul runs in fp8 e4m3 DoubleRow mode
(213ns per 512-col matmul warm = 512 cycles @2.4GHz streaming 2 packed
rhs cols/cycle; 2x bf16 FLOPs via 256-deep contraction). W1b is
host-scaled by 32 so its N(0, 1/1024) entries use the e4m3 range; the
tanh compensates via the ACT scale=1/32 immediate.

v20 changes (from trace analysis of the 75.2us v19 baseline):
  - DMA issue is SP-queue rate-limited (~650ns per DMA_DIRECT2D): merge
    to 11 host-packed loads in strict first-need order (x0, w1a, w1b,
    xi q0, xi q1, bmask, w0, xi q2, xi q3, w2). w1a first => phase 1
    runs ~10-13us instead of 16-19us.
  - Phase 1 rework: h0T via 8 [16,512] matmuls (+8 concurrent
    tile_position=(0,32) duplicates for the row-32 bias-burst replica),
    then h0 [128h,b]-layout via 8 one-hot K=16 transpose-matmuls
    against a host identity, replacing v19's 32 tiny matmuls (saves
    ~4us PE and pulls the first ACT call ~5us earlier).
  - S4 bias one-hot matmuls run as 2-tile row-group bursts
    (tile_position=(0,0)/(32,0)): concurrent in the PE array, ~halving
    the 0.63us/tile bias cost. Requires h0T+bmask replicas at
    partitions 32-47.
  - Tiles scheduled as (h, q-pair)s; per-pair class (s1 = 4 fused-bias
    ACT calls, s4 = PE bias + 1 big ACT call) balances PE vs ACT:
    ACT small call ~590ns, big 1024-col ~1040ns, DVE reduce ~1210ns.

All DRAM tensors are host-packed so every SBUF tile loads with ONE
contiguous dma_start:
  xiQ4  [4*128, 4096] fp8   row q*128+p, col fp*2048+i*1024+c
  w1bQ2 [128, 4096]   fp8   (= 32*W1b.T, DoubleRow pair layout)
  x0T   [128, 4*16]   fp16  host-packed f-chunks side by side
  x0Q8  [128, 4*16]   fp8   same, for the fp8 W0-term matmuls
  w1a4  [128, 4096]   fp16  (= W1a.T, f-chunks side by side)
  w2Q4  [128, 4096]   fp16  h-tile pairs side by side (= W2.T regrouped)
  w0Q   [128, 2048]   fp8   f-chunks side by side (= W0.T regrouped)
  bmask [48, 4*1024]  fp16  one-hot bias mask; rows 32-47 replicate 0-15
  id16  [16, 16]      fp16  identity (h0 layout transpose)
Output res [BL=16, F=512] per core (batch-major); host concatenates.
"""

import os
import numpy as np
from contextlib import ExitStack

import concourse.bass as bass
import concourse.tile as tile
from concourse import bacc, mybir
from concourse.bass_utils import run_bass_kernel_spmd

N_CORES = 8
B, N, F, H = 128, 256, 512, 1024
BL = B // N_CORES          # 16 batches per core
NI = N - 1                 # 255 real columns per batch
NP = 256                   # padded columns per batch
NF = F // 128              # 4 f-chunks
FP = 2                     # 2 f-pair chunks (256 rows each, DoubleRow)
NH = H // 128              # 8 h-tiles
QUADS = BL // 4            # 4 batch-quads; per quad psum tile [128, 4*256]
QW = 4 * NP                # 1024 columns per quad
WSCALE = 32.0              # host-side W1b/bias scale (ACT scale=1/32)

F32 = mybir.dt.float32
BF16 = mybir.dt.bfloat16
F16 = mybir.dt.float16
F8 = mybir.dt.float8e4
DR = mybir.MatmulPerfMode.DoubleRow

# Knobs (sweepable on hw):
#  KB_NS1: number of s1 PAIRS (2 tiles each). Rest are s4 pairs.
#  KB_BURST: 1 = s4 bias matmuls as 2-tile row-group bursts; 0 = serial.
#  KB_DUP: 1 = h0T replica via concurrent tile_position=(0,32) phase1b
#          duplicate; 0 = serial second pass.
#  KB_NGH: consumes prefaced by a GpSimd halving add (measured 1154ns on
#          hw per tile = 0.37 eff), then a half-width DVE reduce.
#  KB_WARM: dummy [128,128] f32 matmuls (426ns each) to hold the PE
#          pstate ramp until the first real operands land.
#  KB_PPB: main PSUM pool bufs ([128,1024] f32 slots, 2 banks each).
NS1P = int(os.environ.get("KB_NS1", "8"))
BURST = int(os.environ.get("KB_BURST", "1"))
DUP = int(os.environ.get("KB_DUP", "1"))
NGH = int(os.environ.get("KB_NGH", "0"))
WARM_N = int(os.environ.get("KB_WARM", "12"))
PPB = int(os.environ.get("KB_PPB", "4"))
TAIL_S4 = int(os.environ.get("KB_TAIL", "3"))
ITB = int(os.environ.get("KB_ITB", "8"))


def _build_kernel():
    nc = bacc.Bacc(
        "TRN2", target_bir_lowering=False, debug=False, num_devices=N_CORES
    )

    xiQ4 = nc.dram_tensor("xiQ4", [QUADS * 128, 4096], F8, kind="ExternalInput").ap()
    w1bQ2 = nc.dram_tensor("w1bQ2", [128, 4096], F8, kind="ExternalInput").ap()
    x0T = nc.dram_tensor("x0T", [128, NF * BL], F16, kind="ExternalInput").ap()
    x0Q8 = nc.dram_tensor("x0Q8", [128, NF * BL], F8, kind="ExternalInput").ap()
    w1a4 = nc.dram_tensor("w1a4", [128, 4096], F16, kind="ExternalInput").ap()
    w2Q4 = nc.dram_tensor("w2Q4", [128, 4096], F16, kind="ExternalInput").ap()
    w0Q = nc.dram_tensor("w0Q", [128, 2048], F8, kind="ExternalInput").ap()
    bmaskT = nc.dram_tensor("bmaskT", [48, QUADS * 1024], F16, kind="ExternalInput").ap()
    id16 = nc.dram_tensor("id16", [16, 16], F16, kind="ExternalInput").ap()
    res = nc.dram_tensor("res", [BL, F], F32, kind="ExternalOutput").ap()

    with tile.TileContext(nc) as tc:
        with ExitStack() as ctx:
            _kernel_body(
                ctx, tc, xiQ4, w1bQ2, x0T, x0Q8, w1a4, w2Q4, w0Q, bmaskT,
                id16, res
            )

    nc.compile()
    return nc


def _kernel_body(ctx, tc, xiQ4, w1bQ2, x0T, x0Q8, w1a4, w2Q4, w0Q, bmaskT,
                 id16, res):
    nc = tc.nc
    Tanh = mybir.ActivationFunctionType.Tanh

    wpool = ctx.enter_context(tc.tile_pool(name="weights", bufs=1))

    def load_rows(name, dram, shape, dt):
        t = wpool.tile(shape, dt, tag=name, name=name)
        flat = t[:] if len(shape) == 2 else t[:].rearrange(
            "p a b c -> p (a b c)" if len(shape) == 4 else "p a b -> p (a b)"
        )
        nc.sync.dma_start(flat, dram[0 : shape[0], :])
        return t

    def load_rows_at(name, dram, r0, shape, dt):
        t = wpool.tile(shape, dt, tag=name, name=name)
        flat = t[:].rearrange("p a b c -> p (a b c)")
        nc.sync.dma_start(flat, dram[r0 : r0 + 128, :])
        return t

    # ---- DMA issue order = first-need order. Each DMA_DIRECT2D costs
    # ~650ns serialized on the SP queue, so the count and order ARE the
    # lead-in: w1a gates phase 1 (which gates every ACT call), xi q0
    # gates the first tile's matmuls.
    x0_all = load_rows("x0", x0T, [128, NF * BL], F16)
    w1a_all = load_rows("w1a", w1a4, [128, 4, 1024], F16)
    w1b2 = load_rows("w1b", w1bQ2, [128, 2, 2, 1024], F8)
    xi_sb = [None] * QUADS
    for q in (0, 1):
        xi_sb[q] = load_rows_at(f"xi_{q}", xiQ4, q * 128, [128, 2, 2, 1024], F8)
    # tiny loads out of the head: small-row DMAs ahead of w1a delayed the
    # first packets ~2us (descriptor inefficiency); id16 is needed only by
    # the one-hot flip (~17us), x0Q8 only by the W0 epilogue (~29us).
    id_sb = wpool.tile([16, 16], F16, tag="id16", name="id16")
    nc.sync.dma_start(id_sb[:], id16[:, :])
    x08_all = load_rows("x08", x0Q8, [128, NF * BL], F8)
    bmask_sb = wpool.tile([48, QUADS * 1024], F16, tag="bmask", name="bmask")
    nc.sync.dma_start(bmask_sb[:], bmaskT[:, :])
    w0_sb = load_rows("w0", w0Q, [128, 2048], F8)
    for q in (2, 3):
        xi_sb[q] = load_rows_at(f"xi_{q}", xiQ4, q * 128, [128, 2, 2, 1024], F8)
    w2_all = load_rows("w2", w2Q4, [128, 4, 1024], F16)

    x0_sb = [x0_all[:, f * BL : (f + 1) * BL] for f in range(NF)]
    x08_sb = [x08_all[:, f * BL : (f + 1) * BL] for f in range(NF)]

    def w2_slice(h):
        return w2_all[:, h // 2, (h % 2) * 512 : (h % 2 + 1) * 512]

    # h0T2: rows 0-15 = h0T [b, h], rows 32-47 = replica for row-group
    # bias bursts. h0_all: [128, h*16+b] fp16 for the ACT bias port.
    h0T2_sb = wpool.tile([48, H], F16, tag="h0T", name="h0T")
    h0_all = wpool.tile([128, NH * BL], F16, tag="h0a", name="h0a")
    S_sb = [
        wpool.tile([128, BL], F16, tag=f"S_{h}", name=f"S_{h}")
        for h in range(NH)
    ]

    # One PSUM pool; every tile shares the tag so slots recycle.
    # Slot = [128, 4*NP] f32 = 2 banks; PPB slots = the full 8 banks.
    ppool = ctx.enter_context(tc.tile_pool(name="ps", bufs=PPB, space="PSUM"))
    itpool = ctx.enter_context(tc.tile_pool(name="it", bufs=ITB))

    # ---- Phase 0: PE warm-up to ride the pstate ramp until w1a lands.
    if WARM_N:
        wz = wpool.tile([128, 128], F32, tag="warmz", name="warmz")
        nc.vector.memset(wz[:], 0.0)
        pw = ppool.tile([128, 128], F32, tag="ps", name="pwarm")
        for _ in range(WARM_N):
            nc.tensor.matmul(pw[:], wz[:], wz[:], start=True, stop=True)

    # ---- Phase 0b: preload the tanh ACT table during the DMA lead-in
    tiny = wpool.tile([128, 1], F32, tag="tiny", name="tiny")
    nc.vector.memset(tiny[:], 0.0)
    nc.scalar.activation(tiny[:], tiny[:], Tanh)

    # ---- Phase 1: h0T[b,h] = sum_f x0[b,f] W1a[h,f] via 8 [16,512]
    # matmuls (f-outer so each starts as its w1a chunk lands), plus a
    # concurrent col-group-32 duplicate for the burst replica. Then
    # h0[h*128+p, b] via 8 one-hot K=16 matmuls against id16.
    def phase1():
        ph = ppool.tile([48, H], F32, tag="ps", name="ph_h0T")
        for f in range(NF):
            for hb in range(2):
                nc.tensor.matmul(
                    ph[0:BL, hb * 512 : (hb + 1) * 512],
                    x0_sb[f],
                    w1a_all[:, f, hb * 512 : (hb + 1) * 512],
                    start=(f == 0),
                    stop=(f == NF - 1),
                )
                if DUP:
                    nc.tensor.matmul(
                        ph[32:48, hb * 512 : (hb + 1) * 512],
                        x0_sb[f],
                        w1a_all[:, f, hb * 512 : (hb + 1) * 512],
                        start=(f == 0),
                        stop=(f == NF - 1),
                        tile_position=(0, 32),
                        skip_group_check=True,
                    )
        with nc.allow_low_precision(reason="h0T feeds fp16 bias matmul"):
            nc.vector.tensor_copy(h0T2_sb[0:BL, :], ph[0:BL, :])
            if DUP:
                nc.vector.tensor_copy(h0T2_sb[32:48, :], ph[32:48, :])
        if not DUP:
            for f in range(NF):
                for hb in range(2):
                    nc.tensor.matmul(
                        ph[32:48, hb * 512 : (hb + 1) * 512],
                        x0_sb[f],
                        w1a_all[:, f, hb * 512 : (hb + 1) * 512],
                        start=(f == 0),
                        stop=(f == NF - 1),
                    )
            with nc.allow_low_precision(reason="h0T replica"):
                nc.vector.tensor_copy(h0T2_sb[32:48, :], ph[32:48, :])
        # h0 layout flip: psum[128, h*16+b] = h0T[0:16, h*128+p].T @ id16
        ph0 = ppool.tile([128, NH * BL], F32, tag="ps", name="ph_h0")
        for h in range(NH):
            nc.tensor.matmul(
                ph0[:, h * BL : (h + 1) * BL],
                h0T2_sb[0:BL, h * 128 : (h + 1) * 128],
                id_sb[:],
                start=True,
                stop=True,
            )
        with nc.allow_low_precision(reason="h0 bias in fp16 like h0T"):
            nc.vector.tensor_copy(h0_all[:], ph0[:])

    # ---- Phase 3: epilogue res = W0 x0 + W2 S, accumulated in SBUF.
    rt_acc = wpool.tile([BL, F], F32, tag="rt", name="rt_acc")

    def epilogue_w0():
        pw = ppool.tile([BL, F], F32, tag="ps", name="po_w0")
        for f in range(NF):
            nc.tensor.matmul(
                pw[:],
                x08_sb[f],
                w0_sb[:, f * 512 : (f + 1) * 512],
                start=(f == 0),
                stop=(f == NF - 1),
            )
        nc.vector.tensor_copy(rt_acc[:], pw[:])

    def epilogue_s_group(hs, name):
        pg = ppool.tile([BL, F], F32, tag="ps", name=name)
        for i, h in enumerate(hs):
            nc.tensor.matmul(
                pg[:], S_sb[h][:], w2_slice(h),
                start=(i == 0), stop=(i == len(hs) - 1),
            )
        nc.vector.tensor_add(rt_acc[:], rt_acc[:], pg[:])

    # ---- Phase 2: hi matmul (fp8 DoubleRow) + bias + tanh + reduce ----
    def consume(h, q, pb, cls, red):
        it = itpool.tile([128, 4 * NP], BF16, tag="it", name=f"it_{h}_{q}")
        s1ish = cls in ("s1", "s1a")
        nb = NP if cls == "s4" else NI
        with nc.allow_low_precision(
            reason="S accumulated in 16-bit to feed the 16-bit output matmul"
        ):
            if s1ish:
                for bl in range(4):
                    b = q * 4 + bl
                    acc = S_sb[h][:, b : b + 1] if cls == "s1a" else None
                    nc.scalar.activation(
                        it[:, bl * NP : bl * NP + NI],
                        pb[:, bl * NP : bl * NP + NI],
                        Tanh,
                        bias=h0_all[:, h * BL + b : h * BL + b + 1],
                        scale=1.0 / WSCALE,
                        accum_out=acc,
                    )
                if cls == "s1a":
                    return
            else:
                # S4: bias already in PSUM (one-hot matmul, pad col exact 0
                # since bmask zeroes it and tanh(0)=0): one big tanh call.
                nc.scalar.activation(it[:], pb[:], Tanh, scale=1.0 / WSCALE)
            scol = S_sb[h][:, q * 4 : (q + 1) * 4]
            view = it[:].rearrange("p (b n) -> p b n", b=4)
            if red == "gph":
                hb = nb // 2
                nc.gpsimd.tensor_add(
                    view[:, :, :hb],
                    view[:, :, :hb],
                    view[:, :, nb - hb : nb],
                )
                nc.vector.reduce_sum(
                    scol, view[:, :, : nb - hb], axis=mybir.AxisListType.X
                )
            else:
                nc.vector.reduce_sum(
                    scol, view[:, :, :nb], axis=mybir.AxisListType.X
                )

    def mm_main(pb, h, q, s1ish):
        # 2 DoubleRow matmuls per 512-col block: fpair 0 starts, fpair 1
        # accumulates; S4 groups stay open for the bias matmul.
        for bk in range(2):
            out = pb[:, bk * 512 : (bk + 1) * 512]
            for fp in range(FP):
                nc.tensor.matmul(
                    out,
                    w1b2[:, fp, :, h * 128 : (h + 1) * 128],
                    xi_sb[q][:, fp, :, bk * 512 : (bk + 1) * 512],
                    start=(fp == 0),
                    stop=(fp == FP - 1) and s1ish,
                    perf_mode=DR,
                )

    def mm_bias(pb, h, q, row):
        # One-hot bias matmul; row-group `row` (0 or 32) lets two of
        # these run concurrently in the PE array.
        for bk in range(2):
            nc.tensor.matmul(
                pb[:, bk * 512 : (bk + 1) * 512],
                h0T2_sb[row : row + BL, h * 128 : (h + 1) * 128],
                bmask_sb[row : row + BL,
                         q * 1024 + bk * 512 : q * 1024 + (bk + 1) * 512],
                start=False,
                stop=True,
                tile_position=(row, 0) if BURST else None,
            )

    def mm_bias_burst(pb0, pb1, h, q0, q1):
        # Interleave the two tiles' bias mms bk-wise so the row-0 and
        # row-32 instructions sit back-to-back and overlap in the array.
        for bk in range(2):
            for row, pb, q in ((0, pb0, q0), (32, pb1, q1)):
                nc.tensor.matmul(
                    pb[:, bk * 512 : (bk + 1) * 512],
                    h0T2_sb[row : row + BL, h * 128 : (h + 1) * 128],
                    bmask_sb[row : row + BL,
                             q * 1024 + bk * 512 : q * 1024 + (bk + 1) * 512],
                    start=False,
                    stop=True,
                    tile_position=(row, 0),
                )

    # ---- Schedule: 16 (h, q-pair)s. Wave 0 ascends h on quads (0,1);
    # wave 1 descends h on quads (2,3) so S[7..4] complete early and
    # their epilogue group issues mid-stream.
    pairs = []
    for wave in range(2):
        hs = range(NH) if wave == 0 else range(NH - 1, -1, -1)
        for h in hs:
            pairs.append((h, 2 * wave, 2 * wave + 1))

    # Deficit-spread pair classes; last TAIL_S4 pairs forced s4 (single
    # big tanh call drains the ACT pipeline fastest).
    counts = {"s1": NS1P, "s4": 16 - NS1P}
    labels = []
    used = {k: 0 for k in counts}
    for pos in range(16):
        opts = [k for k in counts if used[k] < counts[k]]
        if pos == 0 and counts["s1"]:
            # pair 0 stays s1: its ACT needs only h0_all, not the longer
            # h0T-copy -> bias-matmul chain (lead-in critical path).
            opts = ["s1"]
        elif pos >= 16 - TAIL_S4 and used["s4"] < counts["s4"]:
            opts = ["s4"]
        pick = max(opts, key=lambda k: counts[k] * (pos + 1) / 16 - used[k])
        used[pick] += 1
        labels.append(pick)
    nred = sum(2 for l in labels)
    rcounts = {"gph": min(NGH, nred)}
    rcounts["plain"] = nred - rcounts["gph"]
    rlabels = []
    rused = {k: 0 for k in rcounts}
    for pos in range(nred):
        opts = [k for k in rcounts if rused[k] < rcounts[k]]
        if pos >= nred - 4 and rused["plain"] < rcounts["plain"]:
            opts = ["plain"]
        pick = max(opts, key=lambda k: rcounts[k] * (pos + 1) / nred - rused[k])
        rused[pick] += 1
        rlabels.append(pick)

    phase1()

    for pos, (h, qa, qb) in enumerate(pairs):
        cls = labels[pos]
        reda, redb = rlabels[2 * pos], rlabels[2 * pos + 1]
        pba = ppool.tile([128, 4 * NP], F32, tag="ps", name=f"pb_{h}_{qa}")
        pbb = ppool.tile([128, 4 * NP], F32, tag="ps", name=f"pb_{h}_{qb}")
        # Interleaved mains (fp-outer, qa/qb alternating) so both tiles
        # finish together and the 4 bias matmuls sit adjacent in the PE
        # stream, where the row-0/row-32 pairs overlap in the array.
        for fp in range(FP):
            for bk in range(2):
                for pb, q in ((pba, qa), (pbb, qb)):
                    nc.tensor.matmul(
                        pb[:, bk * 512 : (bk + 1) * 512],
                        w1b2[:, fp, :, h * 128 : (h + 1) * 128],
                        xi_sb[q][:, fp, :, bk * 512 : (bk + 1) * 512],
                        start=(fp == 0),
                        stop=(fp == FP - 1) and cls != "s4",
                        perf_mode=DR,
                    )
        if cls == "s4":
            mm_bias_burst(pba, pbb, h, qa, qb)
        consume(h, qa, pba, cls, reda)
        consume(h, qb, pbb, cls, redb)
        if pos == 5:
            epilogue_w0()
        if pos == 12:
            epilogue_s_group([7, 6, 5, 4], "po_sA")
        if pos == 14:
            epilogue_s_group([3, 2], "po_sB1")

    epilogue_s_group([1, 0], "po_sB2")
    nc.sync.dma_start(res[:], rt_acc[:])


_NC_CACHE = {}


def _get_nc():
    key = ("v24", NS1P, BURST, DUP, NGH, WARM_N, PPB, TAIL_S4, ITB)
    if key not in _NC_CACHE:
        _NC_CACHE[key] = _build_kernel()
    return _NC_CACHE[key]


def _make_in_maps(x, W1, W2, W0):
    import ml_dtypes

    f8 = ml_dtypes.float8_e4m3
    x = np.ascontiguousarray(np.asarray(x, dtype=np.float32))
    W1 = np.asarray(W1, dtype=np.float32)
    W2 = np.asarray(W2, dtype=np.float32)
    W0 = np.asarray(W0, dtype=np.float32)

    w1aT = np.ascontiguousarray(W1[:, :F].T).astype(np.float16)       # [F, H]
    w1a4 = np.ascontiguousarray(
        w1aT.reshape(NF, 128, H).transpose(1, 0, 2).reshape(128, NF * H)
    )
    w1bT = (W1[:, F:].T * WSCALE).astype(f8)                          # [F, H]
    # DoubleRow pair layout: [128, fp*2048 + i*1024 + h]
    w1bQ2 = np.ascontiguousarray(
        w1bT.reshape(FP, 2, 128, H).transpose(2, 0, 1, 3).reshape(128, 4 * H)
    )
    w2T = np.ascontiguousarray(W2.T).astype(np.float16)               # [H, F]
    w2Q4 = np.ascontiguousarray(
        w2T.reshape(NF, 2, 128, F).transpose(2, 0, 1, 3).reshape(128, NF * 2 * F)
    )
    w0T = np.ascontiguousarray(W0.T).astype(f8)                       # [F, F]
    w0Q = np.ascontiguousarray(
        w0T.reshape(NF, 128, F).transpose(1, 0, 2).reshape(128, NF * F)
    )

    # bmask[r, q*1024 + b*256 + n] = WSCALE iff r%32 == q*4+b and n != 255;
    # rows 32-47 replicate rows 0-15 for the row-group bias bursts.
    bmask = np.zeros((48, QUADS, 4, NP), dtype=np.float16)
    for qq in range(QUADS):
        for bb in range(4):
            bmask[qq * 4 + bb, qq, bb, :NI] = WSCALE
            bmask[32 + qq * 4 + bb, qq, bb, :NI] = WSCALE
    bmask = bmask.reshape(48, QUADS * 1024)

    id16 = np.eye(16, dtype=np.float16)

    in_maps = []
    for i in range(N_CORES):
        xc = x[i * BL : (i + 1) * BL]               # [BL, N, F]
        # packed [128, NF*BL]: row p, block f holds x0T[f*128+p, :]
        x0p = np.ascontiguousarray(
            xc[:, 0, :].T.reshape(NF, 128, BL).transpose(1, 0, 2).reshape(128, NF * BL)
        )
        pad = np.zeros((BL, NP, F), dtype=np.float32)
        pad[:, :NI, :] = xc[:, 1:, :]
        xiT = pad.reshape(BL * NP, F).T.astype(f8)  # [F, BL*NP]
        # row q*128+p, col fp*2048 + i*1024 + c
        xiQ4 = np.ascontiguousarray(
            xiT.reshape(FP, 2, 128, QUADS, QW)
            .transpose(3, 2, 0, 1, 4)
            .reshape(QUADS * 128, 4 * QW)
        )
        in_maps.append(
            {
                "xiQ4": xiQ4,
                "x0T": x0p.astype(np.float16),
                "x0Q8": x0p.astype(f8),
                "w1bQ2": w1bQ2,
                "w1a4": w1a4,
                "w2Q4": w2Q4,
                "w0Q": w0Q,
                "bmaskT": bmask,
                "id16": id16,
            }
        )
    return in_maps


def _gather(results):
    out = np.empty((B, F), dtype=np.float32)
    for i in range(N_CORES):
        out[i * BL : (i + 1) * BL] = results[i]["res"]
    return out


def kernel(x, W1, W2, W0):
    nc = _get_nc()
    in_maps = _make_in_maps(x, W1, W2, W0)
    res = run_bass_kernel_spmd(nc, in_maps, list(range(N_CORES)))
    return _gather(res.results)


def kernel_profiled(x, W1, W2, W0, **trace_kwargs):
    """Like kernel() but with NTFF profiling; returns (out, exec_time_ns)."""
    nc = _get_nc()
    in_maps = _make_in_maps(x, W1, W2, W0)
    res = run_bass_kernel_spmd(
        nc, in_maps, list(range(N_CORES)), trace=True, **trace_kwargs
    )
    return _gather(res.results), res.exec_time_ns


# revision 19
# speedup vs baseline: 1.0551x; 1.0044x over previous
"""Trainium2 Bass kernel for nn_Attention_39934605918652.

res[b] = W0 @ x0[b] + sum_{n=1..N-1} W2 @ tanh(W1a @ x0[b] + W1b @ x[b,n])

Key algebraic optimization: W2 does not depend on n, so
    sum_n W2 @ tanh(...) = W2 @ (sum_n tanh(...))
which removes the second big matmul (only a [B,H]x[H,F] remains).

Sharding: data-parallel over batch B=128 across 8 cores (16 batches/core),
weights replicated. No collectives.

The dominant [F=512]-contraction matmul runs in fp8 e4m3 DoubleRow mode
(213ns per 512-col matmul warm = 512 cycles @2.4GHz streaming 2 packed
rhs cols/cycle; 2x bf16 FLOPs via 256-deep contraction). W1b is
host-scaled by 32 so its N(0, 1/1024) entries use the e4m3 range; the
tanh compensates via the ACT scale=1/32 immediate.

v20 changes (from trace analysis of the 75.2us v19 baseline):
  - DMA issue is SP-queue rate-limited (~650ns per DMA_DIRECT2D): merge
    to 11 host-packed loads in strict first-need order (x0, w1a, w1b,
    xi q0, xi q1, bmask, w0, xi q2, xi q3, w2). w1a first => phase 1
    runs ~10-13us instead of 16-19us.
  - Phase 1 rework: h0T via 8 [16,512] matmuls (+8 concurrent
    tile_position=(0,32) duplicates for the row-32 bias-burst replica),
    then h0 [128h,b]-layout via 8 one-hot K=16 transpose-matmuls
    against a host identity, replacing v19's 32 tiny matmuls (saves
    ~4us PE and pulls the first ACT call ~5us earlier).
  - S4 bias one-hot matmuls run as 2-tile row-group bursts
    (tile_position=(0,0)/(32,0)): concurrent in the PE array, ~halving
    the 0.63us/tile bias cost. Requires h0T+bmask replicas at
    partitions 32-47.
  - Tiles scheduled as (h, q-pair)s; per-pair class (s1 = 4 fused-bias
    ACT calls, s4 = PE bias + 1 big ACT call) balances PE vs ACT:
    ACT small call ~590ns, big 1024-col ~1040ns, DVE reduce ~1210ns.

All DRAM tensors are host-packed so every SBUF tile loads with ONE
contiguous dma_start:
  xiQ4  [4*128, 4096] fp8   row q*128+p, col fp*2048+i*1024+c
  w1bQ2 [128, 4096]   fp8   (= 32*W1b.T, DoubleRow pair layout)
  x0T   [128, 4*16]   fp16  host-packed f-chunks side by side
  x0Q8  [128, 4*16]   fp8   same, for the fp8 W0-term matmuls
  w1a4  [128, 4096]   fp16  (= W1a.T, f-chunks side by side)
  w2Q4  [128, 4096]   fp16  h-tile pairs side by side (= W2.T regrouped)
  w0Q   [128, 2048]   fp8   f-chunks side by side (= W0.T regrouped)
  bmask [48, 4*1024]  fp16  one-hot bias mask; rows 32-47 replicate 0-15
  id16  [16, 16]      fp16  identity (h0 layout transpose)
Output res [BL=16, F=512] per core (batch-major); host concatenates.
"""

import os
import numpy as np
from contextlib import ExitStack

import concourse.bass as bass
import concourse.tile as tile
from concourse import bacc, mybir
from concourse.bass_utils import run_bass_kernel_spmd

N_CORES = 8
B, N, F, H = 128, 256, 512, 1024
BL = B // N_CORES          # 16 batches per core
NI = N - 1                 # 255 real columns per batch
NP = 256                   # padded columns per batch
NF = F // 128              # 4 f-chunks
FP = 2                     # 2 f-pair chunks (256 rows each, DoubleRow)
NH = H // 128              # 8 h-tiles
QUADS = BL // 4            # 4 batch-quads; per quad psum tile [128, 4*256]
QW = 4 * NP                # 1024 columns per quad
WSCALE = 32.0              # host-side W1b/bias scale (ACT scale=1/32)

F32 = mybir.dt.float32
BF16 = mybir.dt.bfloat16
F16 = mybir.dt.float16
F8 = mybir.dt.float8e4
DR = mybir.MatmulPerfMode.DoubleRow

# Knobs (sweepable on hw):
#  KB_NS1: number of s1 PAIRS (2 tiles each). Rest are s4 pairs.
#  KB_BURST: 1 = s4 bias matmuls as 2-tile row-group bursts; 0 = serial.
#  KB_DUP: 1 = h0T replica via concurrent tile_position=(0,32) phase1b
#          duplicate; 0 = serial second pass.
#  KB_NGH: consumes prefaced by a GpSimd halving add (measured 1154ns on
#          hw per tile = 0.37 eff), then a half-width DVE reduce.
#  KB_WARM: dummy [128,128] f32 matmuls (426ns each) to hold the PE
#          pstate ramp until the first real operands land.
#  KB_PPB: main PSUM pool bufs ([128,1024] f32 slots, 2 banks each).
NS1P = int(os.environ.get("KB_NS1", "8"))
BURST = int(os.environ.get("KB_BURST", "1"))
DUP = int(os.environ.get("KB_DUP", "1"))
NGH = int(os.environ.get("KB_NGH", "0"))
WARM_N = int(os.environ.get("KB_WARM", "12"))
PPB = int(os.environ.get("KB_PPB", "4"))
TAIL_S4 = int(os.environ.get("KB_TAIL", "3"))
ITB = int(os.environ.get("KB_ITB", "8"))


def _build_kernel():
    nc = bacc.Bacc(
        "TRN2", target_bir_lowering=False, debug=False, num_devices=N_CORES
    )

    xiQ4 = nc.dram_tensor("xiQ4", [QUADS * 128, 4096], F8, kind="ExternalInput").ap()
    w1bQ2 = nc.dram_tensor("w1bQ2", [128, 4096], F8, kind="ExternalInput").ap()
    x0T = nc.dram_tensor("x0T", [128, NF * BL], F16, kind="ExternalInput").ap()
    x0Q8 = nc.dram_tensor("x0Q8", [128, NF * BL], F8, kind="ExternalInput").ap()
    w1a4 = nc.dram_tensor("w1a4", [128, 4096], F16, kind="ExternalInput").ap()
    w2Q4 = nc.dram_tensor("w2Q4", [128, 4096], F16, kind="ExternalInput").ap()
    w0Q = nc.dram_tensor("w0Q", [128, 2048], F8, kind="ExternalInput").ap()
    bmaskT = nc.dram_tensor("bmaskT", [48, QUADS * 1024], F16, kind="ExternalInput").ap()
    id16 = nc.dram_tensor("id16", [16, 16], F16, kind="ExternalInput").ap()
    res = nc.dram_tensor("res", [BL, F], F32, kind="ExternalOutput").ap()

    with tile.TileContext(nc) as tc:
        with ExitStack() as ctx:
            _kernel_body(
                ctx, tc, xiQ4, w1bQ2, x0T, x0Q8, w1a4, w2Q4, w0Q, bmaskT,
                id16, res
            )

    nc.compile()
    return nc


def _kernel_body(ctx, tc, xiQ4, w1bQ2, x0T, x0Q8, w1a4, w2Q4, w0Q, bmaskT,
                 id16, res):
    nc = tc.nc
    Tanh = mybir.ActivationFunctionType.Tanh

    wpool = ctx.enter_context(tc.tile_pool(name="weights", bufs=1))

    def load_rows(name, dram, shape, dt):
        t = wpool.tile(shape, dt, tag=name, name=name)
        flat = t[:] if len(shape) == 2 else t[:].rearrange(
            "p a b c -> p (a b c)" if len(shape) == 4 else "p a b -> p (a b)"
        )
        nc.sync.dma_start(flat, dram[0 : shape[0], :])
        return t

    def load_rows_at(name, dram, r0, shape, dt):
        t = wpool.tile(shape, dt, tag=name, name=name)
        flat = t[:].rearrange("p a b c -> p (a b c)")
        nc.sync.dma_start(flat, dram[r0 : r0 + 128, :])
        return t

    # ---- DMA issue order = first-need order. Each DMA_DIRECT2D costs
    # ~650ns serialized on the SP queue, so the count and order ARE the
    # lead-in: w1a gates phase 1 (which gates every ACT call), xi q0
    # gates the first tile's matmuls.
    x0_all = load_rows("x0", x0T, [128, NF * BL], F16)
    w1a_all = load_rows("w1a", w1a4, [128, 4, 1024], F16)
    w1b2 = load_rows("w1b", w1bQ2, [128, 2, 2, 1024], F8)
    xi_sb = [None] * QUADS
    for q in (0, 1):
        xi_sb[q] = load_rows_at(f"xi_{q}", xiQ4, q * 128, [128, 2, 2, 1024], F8)
    # tiny loads out of the head: small-row DMAs ahead of w1a delayed the
    # first packets ~2us (descriptor inefficiency); id16 is needed only by
    # the one-hot flip (~17us), x0Q8 only by the W0 epilogue (~29us).
    id_sb = wpool.tile([16, 16], F16, tag="id16", name="id16")
    nc.sync.dma_start(id_sb[:], id16[:, :])
    x08_all = load_rows("x08", x0Q8, [128, NF * BL], F8)
    bmask_sb = wpool.tile([48, QUADS * 1024], F16, tag="bmask", name="bmask")
    nc.sync.dma_start(bmask_sb[:], bmaskT[:, :])
    w0_sb = load_rows("w0", w0Q, [128, 2048], F8)
    for q in (2, 3):
        xi_sb[q] = load_rows_at(f"xi_{q}", xiQ4, q * 128, [128, 2, 2, 1024], F8)
    w2_all = load_rows("w2", w2Q4, [128, 4, 1024], F16)

    x0_sb = [x0_all[:, f * BL : (f + 1) * BL] for f in range(NF)]
    x08_sb = [x08_all[:, f * BL : (f + 1) * BL] for f in range(NF)]

    def w2_slice(h):
        return w2_all[:, h // 2, (h % 2) * 512 : (h % 2 + 1) * 512]

    # h0T2: rows 0-15 = h0T [b, h], rows 32-47 = replica for row-group
    # bias bursts. h0_all: [128, h*16+b] fp16 for the ACT bias port.
    h0T2_sb = wpool.tile([48, H], F16, tag="h0T", name="h0T")
    h0_all = wpool.tile([128, NH * BL], F16, tag="h0a", name="h0a")
    S_sb = [
        wpool.tile([128, BL], F16, tag=f"S_{h}", name=f"S_{h}")
        for h in range(NH)
    ]

    # One PSUM pool; every tile shares the tag so slots recycle.
    # Slot = [128, 4*NP] f32 = 2 banks; PPB slots = the full 8 banks.
    ppool = ctx.enter_context(tc.tile_pool(name="ps", bufs=PPB, space="PSUM"))
    itpool = ctx.enter_context(tc.tile_pool(name="it", bufs=ITB))

    # ---- Phase 0: PE warm-up to ride the pstate ramp until w1a lands.
    if WARM_N:
        wz = wpool.tile([128, 128], F32, tag="warmz", name="warmz")
        nc.vector.memset(wz[:], 0.0)
        pw = ppool.tile([128, 128], F32, tag="ps", name="pwarm")
        for _ in range(WARM_N):
            nc.tensor.matmul(pw[:], wz[:], wz[:], start=True, stop=True)

    # ---- Phase 0b: preload the tanh ACT table during the DMA lead-in
    tiny = wpool.tile([128, 1], F32, tag="tiny", name="tiny")
    nc.vector.memset(tiny[:], 0.0)
    nc.scalar.activation(tiny[:], tiny[:], Tanh)

    # ---- Phase 1: h0T[b,h] = sum_f x0[b,f] W1a[h,f] via 8 [16,512]
    # matmuls (f-outer so each starts as its w1a chunk lands), plus a
    # concurrent col-group-32 duplicate for the burst replica. Then
    # h0[h*128+p, b] via 8 one-hot K=16 matmuls against id16.
    def phase1():
        ph = ppool.tile([48, H], F32, tag="ps", name="ph_h0T")
        for f in range(NF):
            for hb in range(2):
                nc.tensor.matmul(
                    ph[0:BL, hb * 512 : (hb + 1) * 512],
                    x0_sb[f],
                    w1a_all[:, f, hb * 512 : (hb + 1) * 512],
                    start=(f == 0),
                    stop=(f == NF - 1),
                )
                if DUP:
                    nc.tensor.matmul(
                        ph[32:48, hb * 512 : (hb + 1) * 512],
                        x0_sb[f],
                        w1a_all[:, f, hb * 512 : (hb + 1) * 512],
                        start=(f == 0),
                        stop=(f == NF - 1),
                        tile_position=(0, 32),
                        skip_group_check=True,
                    )
        with nc.allow_low_precision(reason="h0T feeds fp16 bias matmul"):
            nc.vector.tensor_copy(h0T2_sb[0:BL, :], ph[0:BL, :])
            if DUP:
                nc.vector.tensor_copy(h0T2_sb[32:48, :], ph[32:48, :])
        if not DUP:
            for f in range(NF):
                for hb in range(2):
                    nc.tensor.matmul(
                        ph[32:48, hb * 512 : (hb + 1) * 512],
                        x0_sb[f],
                        w1a_all[:, f, hb * 512 : (hb + 1) * 512],
                        start=(f == 0),
                        stop=(f == NF - 1),
                    )
            with nc.allow_low_precision(reason="h0T replica"):
                nc.vector.tensor_copy(h0T2_sb[32:48, :], ph[32:48, :])
        # h0 layout flip: psum[128, h*16+b] = h0T[0:16, h*128+p].T @ id16
        ph0 = ppool.tile([128, NH * BL], F32, tag="ps", name="ph_h0")
        for h in range(NH):
            nc.tensor.matmul(
                ph0[:, h * BL : (h + 1) * BL],
                h0T2_sb[0:BL, h * 128 : (h + 1) * 128],
                id_sb[:],
                start=True,
                stop=True,
            )
        with nc.allow_low_precision(reason="h0 bias in fp16 like h0T"):
            nc.vector.tensor_copy(h0_all[:], ph0[:])

    # ---- Phase 3: epilogue res = W0 x0 + W2 S, accumulated in SBUF.
    rt_acc = wpool.tile([BL, F], F32, tag="rt", name="rt_acc")

    def epilogue_w0():
        pw = ppool.tile([BL, F], F32, tag="ps", name="po_w0")
        for f in range(NF):
            nc.tensor.matmul(
                pw[:],
                x08_sb[f],
                w0_sb[:, f * 512 : (f + 1) * 512],
                start=(f == 0),
                stop=(f == NF - 1),
            )
        nc.vector.tensor_copy(rt_acc[:], pw[:])

    def epilogue_s_group(hs, name):
        pg = ppool.tile([BL, F], F32, tag="ps", name=name)
        for i, h in enumerate(hs):
            nc.tensor.matmul(
                pg[:], S_sb[h][:], w2_slice(h),
                start=(i == 0), stop=(i == len(hs) - 1),
            )
        nc.vector.tensor_add(rt_acc[:], rt_acc[:], pg[:])

    # ---- Phase 2: hi matmul (fp8 DoubleRow) + bias + tanh + reduce ----
    def consume(h, q, pb, cls, red):
        it = itpool.tile([128, 4 * NP], BF16, tag="it", name=f"it_{h}_{q}")
        s1ish = cls in ("s1", "s1a")
        nb = NP if cls == "s4" else NI
        with nc.allow_low_precision(
            reason="S accumulated in 16-bit to feed the 16-bit output matmul"
        ):
            if s1ish:
                for bl in range(4):
                    b = q * 4 + bl
                    acc = S_sb[h][:, b : b + 1] if cls == "s1a" else None
                    nc.scalar.activation(
                        it[:, bl * NP : bl * NP + NI],
                        pb[:, bl * NP : bl * NP + NI],
                        Tanh,
                        bias=h0_all[:, h * BL + b : h * BL + b + 1],
                        scale=1.0 / WSCALE,
                        accum_out=acc,
                    )
                if cls == "s1a":
                    return
            else:
                # S4: bias already in PSUM (one-hot matmul, pad col exact 0
                # since bmask zeroes it and tanh(0)=0): one big tanh call.
                nc.scalar.activation(it[:], pb[:], Tanh, scale=1.0 / WSCALE)
            scol = S_sb[h][:, q * 4 : (q + 1) * 4]
            view = it[:].rearrange("p (b n) -> p b n", b=4)
            if red == "gph":
                hb = nb // 2
                nc.gpsimd.tensor_add(
                    view[:, :, :hb],
                    view[:, :, :hb],
                    view[:, :, nb - hb : nb],
                )
                nc.vector.reduce_sum(
                    scol, view[:, :, : nb - hb], axis=mybir.AxisListType.X
                )
            else:
                nc.vector.reduce_sum(
                    scol, view[:, :, :nb], axis=mybir.AxisListType.X
                )

    def mm_main(pb, h, q, s1ish):
        # 2 DoubleRow matmuls per 512-col block: fpair 0 starts, fpair 1
        # accumulates; S4 groups stay open for the bias matmul.
        for bk in range(2):
            out = pb[:, bk * 512 : (bk + 1) * 512]
            for fp in range(FP):
                nc.tensor.matmul(
                    out,
                    w1b2[:, fp, :, h * 128 : (h + 1) * 128],
                    xi_sb[q][:, fp, :, bk * 512 : (bk + 1) * 512],
                    start=(fp == 0),
                    stop=(fp == FP - 1) and s1ish,
                    perf_mode=DR,
                )

    def mm_bias(pb, h, q, row):
        # One-hot bias matmul; row-group `row` (0 or 32) lets two of
        # these run concurrently in the PE array.
        for bk in range(2):
            nc.tensor.matmul(
                pb[:, bk * 512 : (bk + 1) * 512],
                h0T2_sb[row : row + BL, h * 128 : (h + 1) * 128],
                bmask_sb[row : row + BL,
                         q * 1024 + bk * 512 : q * 1024 + (bk + 1) * 512],
                start=False,
                stop=True,
                tile_position=(row, 0) if BURST else None,
            )

    def mm_bias_burst(pb0, pb1, h, q0, q1):
        # Interleave the two tiles' bias mms bk-wise so the row-0 and
        # row-32 instructions sit back-to-back and overlap in the array.
        for bk in range(2):
            for row, pb, q in ((0, pb0, q0), (32, pb1, q1)):
                nc.tensor.matmul(
                    pb[:, bk * 512 : (bk + 1) * 512],
                    h0T2_sb[row : row + BL, h * 128 : (h + 1) * 128],
                    bmask_sb[row : row + BL,
                             q * 1024 + bk * 512 : q * 1024 + (bk + 1) * 512],
                    start=False,
                    stop=True,
                    tile_position=(row, 0),
                )

    # ---- Schedule: 16 (h, q-pair)s. Wave 0 ascends h on quads (0,1);
    # wave 1 descends h on quads (2,3) so S[7..4] complete early and
    # their epilogue group issues mid-stream.
    pairs = []
    for wave in range(2):
        hs = range(NH) if wave == 0 else range(NH - 1, -1, -1)
        for h in hs:
            pairs.append((h, 2 * wave, 2 * wave + 1))

    # Deficit-spread pair classes; last TAIL_S4 pairs forced s4 (single
    # big tanh call drains the ACT pipeline fastest).
    if PAT:
        # Explicit pattern: strict s1/s4 alternation for the first 2*NS1P
        # pairs (identical prefix to the NS1P=8 champion), all-s4 tail.
        # Early consecutive s4 pairs are PE-fill-bound back to back and
        # starve ACT during pipeline fill; late ones are absorbed by the
        # ACT backlog while still saving its small-call overhead.
        labels = [
            "s1" if (i % 2 == 0 and i < 2 * NS1P) else "s4"
            for i in range(16)
        ]
    else:
        counts = {"s1": NS1P, "s4": 16 - NS1P}
        labels = []
        used = {k: 0 for k in counts}
        for pos in range(16):
            opts = [k for k in counts if used[k] < counts[k]]
            if pos == 0 and counts["s1"]:
                # pair 0 stays s1: its ACT needs only h0_all, not the
                # longer h0T-copy -> bias-matmul chain.
                opts = ["s1"]
            elif pos >= 16 - TAIL_S4 and used["s4"] < counts["s4"]:
                opts = ["s4"]
            pick = max(
                opts, key=lambda k: counts[k] * (pos + 1) / 16 - used[k]
            )
            used[pick] += 1
            labels.append(pick)
    nred = sum(2 for l in labels)
    rcounts = {"gph": min(NGH, nred)}
    rcounts["plain"] = nred - rcounts["gph"]
    rlabels = []
    rused = {k: 0 for k in rcounts}
    for pos in range(nred):
        opts = [k for k in rcounts if rused[k] < rcounts[k]]
        if pos >= nred - 4 and rused["plain"] < rcounts["plain"]:
            opts = ["plain"]
        pick = max(opts, key=lambda k: rcounts[k] * (pos + 1) / nred - rused[k])
        rused[pick] += 1
        rlabels.append(pick)

    phase1()

    for pos, (h, qa, qb) in enumerate(pairs):
        cls = labels[pos]
        reda, redb = rlabels[2 * pos], rlabels[2 * pos + 1]
        pba = ppool.tile([128, 4 * NP], F32, tag="ps", name=f"pb_{h}_{qa}")
        pbb = ppool.tile([128, 4 * NP], F32, tag="ps", name=f"pb_{h}_{qb}")
        # Interleaved mains (fp-outer, qa/qb alternating) so both tiles
        # finish together and the 4 bias matmuls sit adjacent in the PE
        # stream, where the row-0/row-32 pairs overlap in the array.
        for fp in range(FP):
            for bk in range(2):
                for pb, q in ((pba, qa), (pbb, qb)):
                    nc.tensor.matmul(
                        pb[:, bk * 512 : (bk + 1) * 512],
                        w1b2[:, fp, :, h * 128 : (h + 1) * 128],
                        xi_sb[q][:, fp, :, bk * 512 : (bk + 1) * 512],
                        start=(fp == 0),
                        stop=(fp == FP - 1) and cls != "s4",
                        perf_mode=DR,
                    )
        if cls == "s4":
            mm_bias_burst(pba, pbb, h, qa, qb)
        consume(h, qa, pba, cls, reda)
        consume(h, qb, pbb, cls, redb)
        if pos == 5:
            epilogue_w0()
        if pos == 12:
            epilogue_s_group([7, 6, 5, 4], "po_sA")
        if pos == 14:
            epilogue_s_group([3, 2], "po_sB1")

    epilogue_s_group([1, 0], "po_sB2")
    nc.sync.dma_start(res[:], rt_acc[:])


_NC_CACHE = {}


def _get_nc():
    key = ("v24", NS1P, BURST, DUP, NGH, WARM_N, PPB, TAIL_S4, ITB)
    if key not in _NC_CACHE:
        _NC_CACHE[key] = _build_kernel()
    return _NC_CACHE[key]


def _make_in_maps(x, W1, W2, W0):
    import ml_dtypes

    f8 = ml_dtypes.float8_e4m3
    x = np.ascontiguousarray(np.asarray(x, dtype=np.float32))
    W1 = np.asarray(W1, dtype=np.float32)
    W2 = np.asarray(W2, dtype=np.float32)
    W0 = np.asarray(W0, dtype=np.float32)

    w1aT = np.ascontiguousarray(W1[:, :F].T).astype(np.float16)       # [F, H]
    w1a4 = np.ascontiguousarray(
        w1aT.reshape(NF, 128, H).transpose(1, 0, 2).reshape(128, NF * H)
    )
    w1bT = (W1[:, F:].T * WSCALE).astype(f8)                          # [F, H]
    # DoubleRow pair layout: [128, fp*2048 + i*1024 + h]
    w1bQ2 = np.ascontiguousarray(
        w1bT.reshape(FP, 2, 128, H).transpose(2, 0, 1, 3).reshape(128, 4 * H)
    )
    w2T = np.ascontiguousarray(W2.T).astype(np.float16)               # [H, F]
    w2Q4 = np.ascontiguousarray(
        w2T.reshape(NF, 2, 128, F).transpose(2, 0, 1, 3).reshape(128, NF * 2 * F)
    )
    w0T = np.ascontiguousarray(W0.T).astype(f8)                       # [F, F]
    w0Q = np.ascontiguousarray(
        w0T.reshape(NF, 128, F).transpose(1, 0, 2).reshape(128, NF * F)
    )

    # bmask[r, q*1024 + b*256 + n] = WSCALE iff r%32 == q*4+b and n != 255;
    # rows 32-47 replicate rows 0-15 for the row-group bias bursts.
    bmask = np.zeros((48, QUADS, 4, NP), dtype=np.float16)
    for qq in range(QUADS):
        for bb in range(4):
            bmask[qq * 4 + bb, qq, bb, :NI] = WSCALE
            bmask[32 + qq * 4 + bb, qq, bb, :NI] = WSCALE
    bmask = bmask.reshape(48, QUADS * 1024)

    id16 = np.eye(16, dtype=np.float16)

    in_maps = []
    for i in range(N_CORES):
        xc = x[i * BL : (i + 1) * BL]               # [BL, N, F]
        # packed [128, NF*BL]: row p, block f holds x0T[f*128+p, :]
        x0p = np.ascontiguousarray(
            xc[:, 0, :].T.reshape(NF, 128, BL).transpose(1, 0, 2).reshape(128, NF * BL)
        )
        pad = np.zeros((BL, NP, F), dtype=np.float32)
        pad[:, :NI, :] = xc[:, 1:, :]
        xiT = pad.reshape(BL * NP, F).T.astype(f8)  # [F, BL*NP]
        # row q*128+p, col fp*2048 + i*1024 + c
        xiQ4 = np.ascontiguousarray(
            xiT.reshape(FP, 2, 128, QUADS, QW)
            .transpose(3, 2, 0, 1, 4)
            .reshape(QUADS * 128, 4 * QW)
        )
        in_maps.append(
            {
                "xiQ4": xiQ4,
                "x0T": x0p.astype(np.float16),
                "x0Q8": x0p.astype(f8),
                "w1bQ2": w1bQ2,
                "w1a4": w1a4,
                "w2Q4": w2Q4,
                "w0Q": w0Q,
                "bmaskT": bmask,
                "id16": id16,
            }
        )
    return in_maps


def _gather(results):
    out = np.empty((B, F), dtype=np.float32)
    for i in range(N_CORES):
        out[i * BL : (i + 1) * BL] = results[i]["res"]
    return out


def kernel(x, W1, W2, W0):
    nc = _get_nc()
    in_maps = _make_in_maps(x, W1, W2, W0)
    res = run_bass_kernel_spmd(nc, in_maps, list(range(N_CORES)))
    return _gather(res.results)


def kernel_profiled(x, W1, W2, W0, **trace_kwargs):
    """Like kernel() but with NTFF profiling; returns (out, exec_time_ns)."""
    nc = _get_nc()
    in_maps = _make_in_maps(x, W1, W2, W0)
    res = run_bass_kernel_spmd(
        nc, in_maps, list(range(N_CORES)), trace=True, **trace_kwargs
    )
    return _gather(res.results), res.exec_time_ns
